# revision 1
# baseline (speedup 1.0000x reference)
"""Multi-head attention (qkv pointwise-conv projection + softmax attention)
on 8 Trainium2 NeuronCores.

Problem shapes (hardcoded):
    x:     [B=4, D=512, L=2048] f32
    w_qkv: [3*D=1536, D=512]    f32
    out:   [B, D, L]            f32

Sharding: 2 cores per batch element; each core owns 4 of the 8 heads
(tensor-parallel on the qkv output channels). Core c -> batch c//2,
head group c%2 (heads 4*(c%2) .. 4*(c%2)+3).

Per-core kernel (all in bf16 compute, f32 accumulate):
    Q/K proj:  q[o,l] = sum_d w[o,d] x[d,l]   (layout [head_dim, L])
    V proj  :  vT[l,o]                          (layout [L, head_dim])
               vT stored per head with a fused ones-column -> attn@[v|1]
               yields both the weighted values and the softmax denominator.
    scores  :  St[j,i] = sum_d k[d,j] q[d,i]  (two heads packed in the
               128-row PE array via row tiling: head0 partitions 0-63,
               head1 partitions 64-127)
    softmax :  exp on ScalarE (scale folded into the activation), no max
               subtraction (scores are O(1) by construction)
    attn@v  :  O[d(+den),i] accumulated over j blocks in PSUM
    norm    :  O[d,i] * broadcast(1/den[i])  (broadcast via K=1 matmul)
"""

import os
import numpy as np

B, D, L, H = 4, 512, 2048, 8
HD = D // H  # 64
N_CORES = 8
SCALE = float(D) ** -0.5

# module-level knobs for test.py; harness uses defaults
TRACE = False
LAST_RESULTS = None

_COMPILED = {}


def _build_nc():
    from contextlib import ExitStack

    import concourse.bass as bass
    import concourse.mybir as mybir
    import concourse.tile as tile
    from concourse.bacc import Bacc

    F32 = mybir.dt.float32
    BF16 = mybir.dt.bfloat16
    Exp = mybir.ActivationFunctionType.Exp

    # Bacc (not plain Bass): its finalize() runs the legalization passes that
    # split multi-wait matmuls (walrus MM struct supports only 1 sync wait).
    nc = Bacc("TRN2", target_bir_lowering=False, debug=False)
    # host pre-permuted layouts -> fully contiguous DMA descriptors (4-6KB)
    # x: [p, lc, dc, l'] where d = dc*128+p, l = lc*512+l'
    x_d = nc.dram_tensor("x", [128, 4, 4, 512], BF16, kind="ExternalInput")
    # wT split q|k vs v: [p, dc, o] where d = dc*128+p
    wqk_d = nc.dram_tensor("wqkT", [128, 4, 512], BF16, kind="ExternalInput")
    wv_d = nc.dram_tensor("wvT", [128, 4, 256], BF16, kind="ExternalInput")
    out_d = nc.dram_tensor("out", [256, L], F32, kind="ExternalOutput")

    NJB = L // 128  # 16 key blocks
    NIC = L // 512  # 4 query chunks

    with ExitStack() as ctx:
        tc = ctx.enter_context(tile.TileContext(nc))
        const = ctx.enter_context(tc.tile_pool(name="const", bufs=1))
        qkp = ctx.enter_context(tc.tile_pool(name="qkp", bufs=1))
        vtp = ctx.enter_context(tc.tile_pool(name="vtp", bufs=1))
        sx = ctx.enter_context(tc.tile_pool(name="sx", bufs=4))
        nrm = ctx.enter_context(tc.tile_pool(name="nrm", bufs=4))
        outp = ctx.enter_context(tc.tile_pool(name="outp", bufs=4))
        drp = ctx.enter_context(tc.tile_pool(name="drp", bufs=4, space="DRAM"))
        ps_st = ctx.enter_context(tc.tile_pool(name="ps_st", bufs=2, space="PSUM"))
        ps_o = ctx.enter_context(tc.tile_pool(name="ps_o", bufs=4, space="PSUM"))

        # ---- PE warmup + load inputs ----
        # ~12 matmuls on zeros keep the PE busy through the input-DMA window
        # so the HAM clock gate opens (1.2 -> 2.4 GHz) before real work.
        scr_sb = const.tile([128, 512], BF16, tag="scr")
        nc.vector.memset(scr_sb[:], 0.0)
        warm_ps = ps_st.tile([128, 1024], F32, tag="st", name="warm")
        for _ in range(12):
            nc.tensor.matmul(warm_ps[:, 0:512], scr_sb[:, 0:128], scr_sb[:])
        # everything on the fast sync (HWDGE) ring, ordered so the first
        # projection group (needs wqk + x chunk 0) can start ASAP
        wqk_sb = const.tile([128, 4, 512], BF16, tag="wqk")
        wv_sb = const.tile([128, 4, 256], BF16, tag="wv")
        x_sb = const.tile([128, 4, 4, 512], BF16, tag="x")
        nc.sync.dma_start(out=wqk_sb[:], in_=wqk_d[:])
        nc.sync.dma_start(out=x_sb[:, 0, :, :], in_=x_d[:, 0, :, :])
        nc.sync.dma_start(out=wv_sb[:], in_=wv_d[:])
        for lc in range(1, 4):
            nc.sync.dma_start(out=x_sb[:, lc, :, :], in_=x_d[:, lc, :, :])
        ones_sb = const.tile([1, 64], F32, tag="ones")
        nc.vector.memset(ones_sb[:], 1.0)


        q_sb = [qkp.tile([128, L], BF16, tag=f"q{p}", name=f"q{p}") for p in range(2)]
        k_sb = [qkp.tile([128, L], BF16, tag=f"k{p}", name=f"k{p}") for p in range(2)]
        vt_sb = [vtp.tile([128, 4, 65], BF16, tag=f"vt{jb}", name=f"vt{jb}") for jb in range(NJB)]

        # Projection groups run in 1-bank [128,512] PSUM tiles from the shared
        # "o" pool so they never contend with the exp-feeding st pipeline.
        def g_qk(p, sec, lc):
            # one 512-wide column group of the Q (sec=0) or K (sec=256)
            # projection for head-pair p
            def f():
                dst = q_sb[p] if sec == 0 else k_sb[p]
                ps = ps_o.tile([128, 512], F32, tag="o", name="projg")
                for dc in range(4):
                    nc.tensor.matmul(
                        ps[:],
                        wqk_sb[:, dc, sec + p * 128 : sec + (p + 1) * 128],
                        x_sb[:, lc, dc, :],
                        start=(dc == 0),
                        stop=(dc == 3),
                    )
                nc.vector.tensor_copy(dst[:, lc * 512 : (lc + 1) * 512], ps[:])

            return f

        def g_vt(jb):
            def f():
                nc.vector.memset(vt_sb[jb][:, :, 64:65], 1.0)
                ps = ps_o.tile([128, 512], F32, tag="o", name="projv")
                for dc in range(4):
                    nc.tensor.matmul(
                        ps[:, 0:256],
                        x_sb[:, jb // 4, dc, (jb % 4) * 128 : (jb % 4 + 1) * 128],
                        wv_sb[:, dc, :],
                        start=(dc == 0),
                        stop=(dc == 3),
                    )
                nc.vector.tensor_copy(
                    vt_sb[jb][:, :, 0:64],
                    ps[:, 0:256].rearrange("par (h e) -> par h e", e=64),
                )

            return f

        def attn_block(p, ic, fillers=(), pe_bcast=False):
            # scores+softmax+attn@v for head pair p, query chunk ic (512 wide)
            # fillers: {jb: [callables]} — projection groups interleaved into
            # the loop to fill PE slack without starving ScalarE
            # pe_bcast: broadcast 1/den on the PE (shorter latency chain) —
            # used for the final block where the chain is the kernel tail
            fillers = dict(fillers)
            i0 = ic * 512

            def st_mms(jb):
                # St[j, i] for both heads of the pair, row-packed in the PE
                st = ps_st.tile([128, 1024], F32, tag="st")
                for hp in range(2):
                    nc.tensor.matmul(
                        st[:, hp * 512 : (hp + 1) * 512],
                        k_sb[p][hp * 64 : (hp + 1) * 64, jb * 128 : (jb + 1) * 128],
                        q_sb[p][hp * 64 : (hp + 1) * 64, i0 : i0 + 512],
                        start=True,
                        stop=True,
                    )
                return st

            o_ps = [ps_o.tile([65, 512], F32, tag="o", name="o_acc") for _ in range(2)]
            st_cur = st_mms(0)
            for jb in range(NJB):
                se = sx.tile([128, 1024], BF16, tag="se")
                nc.scalar.activation(se[:], st_cur[:], Exp, scale=SCALE)
                if jb + 1 < NJB:
                    st_cur = st_mms(jb + 1)
                for f in fillers.get(jb, ()):
                    f()
                for hp in range(2):
                    nc.tensor.matmul(
                        o_ps[hp][:],
                        vt_sb[jb][:, 2 * p + hp, :],
                        se[:, hp * 512 : (hp + 1) * 512],
                        start=(jb == 0),
                        stop=(jb == NJB - 1),
                    )
            # normalize and write out: 1/den on DVE (fast approx), broadcast
            # the row across 64 partitions via a DRAM bounce, multiply.
            for hp in range(2):
                hh = 2 * p + hp
                o = o_ps[hp]
                den_sb = nrm.tile([1, 512], F32, tag="den")
                nc.vector.tensor_copy(den_sb[:], o[64:65, :])
                recip = nrm.tile([1, 512], F32, tag="recip")
                # NB: approx-recip reads garbage from PSUM on HW; SBUF input only
                nc.vector.reciprocal_approx_fast(out=recip[:], in_=den_sb[:])
                rbc = nrm.tile([64, 512], F32, tag="rbc")
                if pe_bcast:
                    bc_ps = ps_o.tile([128, 512], F32, tag="o", name="bcast")
                    nc.tensor.matmul(
                        bc_ps[0:64, :], ones_sb[:], recip[:], start=True, stop=True
                    )
                    nc.vector.tensor_copy(rbc[:], bc_ps[0:64, :])
                else:
                    dbounce = drp.tile([1, 512], F32, tag="db", name="db")
                    nc.sync.dma_start(out=dbounce[:], in_=recip[:])
                    nc.sync.dma_start(
                        out=rbc[:],
                        in_=bass.AP(
                            tensor=dbounce.tensor,
                            offset=dbounce.offset,
                            ap=[[0, 64], [1, 512]],
                        ),
                    )
                ot = outp.tile([64, 512], F32, tag="ot")
                nc.vector.tensor_mul(ot[:], o[0:64, :], rbc[:])
                nc.sync.dma_start(
                    out=out_d[hh * 64 : (hh + 1) * 64, i0 : i0 + 512], in_=ot[:]
                )

        # prologue: just enough projection for the first scores; everything
        # else (vt just-in-time, remaining q0/k0 columns, all of q1/k1) is
        # interleaved so ScalarE starts exp-ing ~6us in and never starves.
        # Constraints: vt[j] before vals(j) of block (0,0); k0 column group m
        # before st(4m); q0 group lc before block (0,lc); q1/k1 before (1,0).
        for f in (g_qk(0, 0, 0), g_qk(0, 256, 0), g_vt(0)):
            f()
        attn_block(0, 0, {
            0: [g_vt(1)],
            1: [g_vt(2), g_qk(0, 256, 1)],
            2: [g_vt(3)],
            3: [g_vt(4)],
            4: [g_vt(5), g_qk(0, 256, 2)],
            5: [g_vt(6)],
            6: [g_vt(7)],
            7: [g_vt(8)],
            8: [g_vt(9), g_qk(0, 256, 3)],
            9: [g_vt(10)],
            10: [g_vt(11)],
            11: [g_vt(12)],
            12: [g_vt(13)],
            13: [g_vt(14)],
            14: [g_vt(15)],
            15: [g_qk(0, 0, 1)],
        })
        attn_block(0, 1, {
            0: [g_qk(0, 0, 2)],
            2: [g_qk(0, 0, 3)],
            6: [g_qk(1, 0, 0)],
            10: [g_qk(1, 256, 0)],
        })
        attn_block(0, 2, {
            0: [g_qk(1, 0, 1)],
            5: [g_qk(1, 256, 1)],
            10: [g_qk(1, 0, 2)],
        })
        attn_block(0, 3, {
            0: [g_qk(1, 256, 2)],
            5: [g_qk(1, 0, 3)],
            10: [g_qk(1, 256, 3)],
        })
        for ic in range(NIC):
            attn_block(1, ic, pe_bcast=(ic == 3))

    nc.finalize()
    return nc


def _get_nc():
    if "nc" not in _COMPILED:
        _COMPILED["nc"] = _build_nc()
    return _COMPILED["nc"]


def _prep_inputs(x, w_qkv):
    """Per-core input maps (host-side sharding)."""
    import ml_dtypes

    bf16 = ml_dtypes.bfloat16
    in_maps = []
    for c in range(N_CORES):
        b, g = c // 2, c % 2
        # x[b] [512, 2048] -> [p, lc, dc, l'] so every DMA descriptor is a
        # 4KB contiguous run
        xb = np.ascontiguousarray(
            x[b].reshape(4, 128, 4, 512).transpose(1, 2, 0, 3)
        ).astype(bf16)
        # w rows for this head group, transposed then laid out [p, dc, o];
        # q|k and v as separate tensors so both DMAs are fully contiguous
        wqk_rows = np.concatenate(
            [
                w_qkv[256 * g : 256 * (g + 1), :],
                w_qkv[512 + 256 * g : 512 + 256 * (g + 1), :],
            ],
            axis=0,
        )  # [512, 512]
        wv_rows = w_qkv[1024 + 256 * g : 1024 + 256 * (g + 1), :]  # [256, 512]
        wqkT = np.ascontiguousarray(
            wqk_rows.T.reshape(4, 128, 512).transpose(1, 0, 2)
        ).astype(bf16)
        wvT = np.ascontiguousarray(
            wv_rows.T.reshape(4, 128, 256).transpose(1, 0, 2)
        ).astype(bf16)
        in_maps.append({"x": xb, "wqkT": wqkT, "wvT": wvT})
    return in_maps


def kernel(x, w_qkv):
    global LAST_RESULTS
    from concourse.bass_utils import run_bass_kernel_spmd

    nc = _get_nc()
    in_maps = _prep_inputs(np.asarray(x), np.asarray(w_qkv))
    res = run_bass_kernel_spmd(
        nc, in_maps, core_ids=list(range(N_CORES)), trace=TRACE
    )
    LAST_RESULTS = res
    out = np.empty((B, D, L), dtype=np.float32)
    for c in range(N_CORES):
        b, g = c // 2, c % 2
        out[b, 256 * g : 256 * (g + 1), :] = res.results[c]["out"]
    return out

